# revision 30
# baseline (speedup 1.0000x reference)
"""Causal self-attention (B=2, S=2048, D=2048, H=16) on 8 trn2 NeuronCores.

Sharding: core c -> batch b = c//4, head-group hg = c%4 (4 heads of 128 dims).
Each core computes its heads' attention plus the partial output projection
(row-parallel split of W_proj); the host sums the 4 partials per batch.

Mixed-precision, fully SBUF-resident pipeline:
 - QKV projection in fp8e4 with DoubleRow matmuls (two 128-deep k-slices
   contracted per instruction); only V rows < 256 are additionally computed
   in bf16 (short-context queries average over few keys, so their V error
   does not wash out). W_qkv is host-scaled by 32 so fp8 weight entries sit
   in the normal range; q/k/v come out scaled by 32 (wp is host-scaled by
   1/32 to compensate).
 - Scores in bf16 (q,k at 32x -> scores at 1024x; the exp activation scale
   folds the 1/1024 back out). exp is batched per k-tile pair as one ACT
   instruction reading a [128,2,512] PSUM strip.
 - attention@V and the softmax-denominator matmuls run fp8 DoubleRow
   everywhere except (q-group 0, k-tiles 0/1), which stays bf16: short-
   context queries are the precision-critical ones; long-context fp8
   error averages out across keys. The denominator uses a ones *matrix*
   as the stationary operand, so it lands PSUM-replicated across all 128
   partitions and needs no broadcast before the reciprocal multiply.
 - Output projection bf16, interleaved per q-group with head 3's
   attention so PE fills the exp-latency gaps; y written bf16 and summed
   on the host in fp32.
"""

import sys

sys.path.insert(0, "/opt/trn_rl_repo")

from contextlib import ExitStack

import ml_dtypes
import numpy as np

import concourse.bass as bass
import concourse.mybir as mybir
import concourse.tile as tile
from concourse import bacc
from concourse.bass_utils import run_bass_kernel_spmd

B, S, D, H = 2, 2048, 2048, 16
HD = D // H  # 128
NH = 4  # heads per core
HG = H // NH  # head groups = 4
P = 128
KT = D // P  # 16 k-tiles over model dim
KJ = KT // 2  # 8 k-pairs for DoubleRow
NQ = 4  # seq blocks of 512
QW = S // NQ  # 512
ST = S // P  # 16 seq tiles of 128
BW = 256  # bf16 window: seq rows [0, BW) use bf16 QKV / e / v
WS = 32.0  # host-side weight scale for fp8
SCALE = float(1.0 / np.sqrt(D).astype(np.float32))
EXP_SCALE = SCALE / (WS * WS)
MASK_NEG = -1.0e9

F32 = mybir.dt.float32
BF16 = mybir.dt.bfloat16
F8 = mybir.dt.float8e4
DR = mybir.MatmulPerfMode.DoubleRow


def build_bass():
    nc = bacc.Bacc("TRN2")

    # fp8 x, pair layout: d = j*256 + i*128 + p  -> [p, j, i, s]
    x8 = nc.declare_dram_parameter("x8", [P, NQ, KJ, 2, QW], F8, isOutput=False)
    # bf16 x, seq rows [0, 256): d = k*128 + p -> [p, k, s]
    xb0 = nc.declare_dram_parameter("xb0", [P, KT, BW], BF16, isOutput=False)
    wq8 = nc.declare_dram_parameter("wq8", [P, KJ, 2, NH * HD], F8, isOutput=False)
    wk8 = nc.declare_dram_parameter("wk8", [P, KJ, 2, NH * HD], F8, isOutput=False)
    wv8 = nc.declare_dram_parameter("wv8", [P, KJ, 2, NH * HD], F8, isOutput=False)
    wvb = nc.declare_dram_parameter("wvb", [P, KT, NH * HD], BF16, isOutput=False)
    wp = nc.declare_dram_parameter("wp", [P, NH, D], BF16, isOutput=False)
    # mask strip: [:, 0:128] triangular (col>=row allowed), [:, 128:384]
    # dead(128) | triangular(128)
    mask = nc.declare_dram_parameter("mask", [P, 3 * P], F32, isOutput=False)
    y = nc.declare_dram_parameter("y", [ST, P, D], BF16, isOutput=True)

    with tile.TileContext(nc) as tc, ExitStack() as top:
        const = top.enter_context(tc.tile_pool(name="const", bufs=1))
        persist = top.enter_context(tc.tile_pool(name="persist", bufs=1))

        mask_sb = const.tile([P, 3 * P], F32)
        nc.sync.dma_start(mask_sb, mask[:, :])
        ones_b = const.tile([P, P], BF16)
        nc.vector.memset(ones_b, 1.0)
        ones8 = const.tile([P, 2, P], F8)
        nc.vector.memset(ones8, 1.0)

        qh = persist.tile([P, NH, S], BF16)  # Q^T per head (32x scale)
        kh = persist.tile([P, NH, S], BF16)  # K^T per head (32x scale)
        v_lo = persist.tile([P, 2, NH * HD], BF16)  # v rows < 256 (32x)
        v8 = persist.tile([P, ST, NH * HD], F8)  # all v rows, fp8 (32x)
        a_sb = persist.tile([P, NH, S], BF16)  # attention out (32x)
        wp_sb = persist.tile([P, NH, D], BF16)  # W_proj slice (1/32 scale)

        ph1a = ExitStack()  # bf16 inputs: freed after the early bf16 matmuls
        ph1 = ExitStack()   # fp8 inputs: freed after the last DR matmul
        ph1_sb = ph1.enter_context(tc.tile_pool(name="ph1sb", bufs=1, side="right"))
        ph1_ps = ph1.enter_context(
            tc.tile_pool(name="ph1ps", bufs=2, space="PSUM", side="right")
        )
        ph1a_sb = ph1a.enter_context(
            tc.tile_pool(name="ph1asb", bufs=1, side="right")
        )

        xb0_sb = ph1a_sb.tile([P, KT, BW], BF16)
        wvb_sb = ph1a_sb.tile([P, KT, NH * HD], BF16)
        x8_sb = ph1_sb.tile([P, NQ, KJ, 2, QW], F8)
        wq8_sb = ph1_sb.tile([P, KJ, 2, NH * HD], F8)
        wk8_sb = ph1_sb.tile([P, KJ, 2, NH * HD], F8)
        wv8_sb = ph1_sb.tile([P, KJ, 2, NH * HD], F8)

        # --- DMA schedule: three queues, ordered by consumption time ---
        # scalar: fp8 q weights, then fp8 x by seq quarter (j-halved for an
        # early first matmul) -- matches the n-major phase-1 loop.
        for j in range(KJ):
            js = slice(j, j + 1)
            nc.scalar.dma_start(wq8_sb[:, js, :, :], wq8[:, js, :, :])
            nc.scalar.dma_start(x8_sb[:, 0, js, :, :], x8[:, 0, js, :, :])
        for n in range(1, NQ):
            nc.scalar.dma_start(x8_sb[:, n, 0:4, :, :], x8[:, n, 0:4, :, :])
            nc.scalar.dma_start(x8_sb[:, n, 4:8, :, :], x8[:, n, 4:8, :, :])
        # gpsimd: fp8 k weights, fp8 v weights, then W_proj
        for j in range(0, KJ, 2):
            js = slice(j, j + 2)
            nc.gpsimd.dma_start(wk8_sb[:, js, :, :], wk8[:, js, :, :])
        nc.gpsimd.dma_start(wv8_sb, wv8[:, :, :, :])
        nc.gpsimd.dma_start(wp_sb, wp[:, :, :])
        # sync: mask, bf16 x block, bf16 v weights (all for the small bf16
        # v pass, consumed ~40us in)
        for kc in range(0, KT, 4):
            ks = slice(kc, kc + 4)
            nc.sync.dma_start(xb0_sb[:, ks, :], xb0[:, ks, :])
            nc.sync.dma_start(wvb_sb[:, ks, :], wvb[:, ks, :])

        def qk_dr(h, n):
            """fp8 DoubleRow q,k for head h, seq block n."""
            cs = slice(h * HD, (h + 1) * HD)
            ss = slice(n * QW, (n + 1) * QW)
            for w_sb, out in ((wq8_sb, qh), (wk8_sb, kh)):
                ps = ph1_ps.tile([P, QW], F32, tag="ps")
                for j in range(KJ):
                    nc.tensor.matmul(
                        ps,
                        lhsT=w_sb[:, j, :, cs],
                        rhs=x8_sb[:, n, j, :, :],
                        start=(j == 0),
                        stop=(j == KJ - 1),
                        perf_mode=DR,
                    )
                nc.vector.tensor_copy(out[:, h, ss], ps)

        def v_block(m):
            """v rows m*128:(m+1)*128 (all 4 heads)."""
            ms = slice(m * P, (m + 1) * P)
            ps = ph1_ps.tile([P, QW], F32, tag="ps")
            if m < 2:
                for k in range(KT):
                    nc.tensor.matmul(
                        ps,
                        lhsT=xb0_sb[:, k, ms],
                        rhs=wvb_sb[:, k, :],
                        start=(k == 0),
                        stop=(k == KT - 1),
                    )
                nc.vector.tensor_copy(v_lo[:, m, :], ps)
                nc.vector.tensor_copy(v8[:, m, :], ps)
            else:
                for j in range(KJ):
                    nc.tensor.matmul(
                        ps,
                        lhsT=x8_sb[:, m // 4, j, :, (m % 4) * P : (m % 4 + 1) * P],
                        rhs=wv8_sb[:, j, :, :],
                        start=(j == 0),
                        stop=(j == KJ - 1),
                        perf_mode=DR,
                    )
                nc.vector.tensor_copy(v8[:, m, :], ps)

        def attn_one_qg(h, qg, pools):
            spool, elpool, ehpool, upool, dpool, rpool = pools
            rs = slice(h * HD, (h + 1) * HD)
            kmax = 4 * qg + 4
            npairs = kmax // 2
            ups = upool.tile([P, QW], F32, tag="u")
            dps = dpool.tile([P, QW], F32, tag="d")
            for j in range(npairs):
                kt0 = 2 * j
                is_d0 = j == 2 * qg
                is_d1 = j == 2 * qg + 1
                c0 = 2 * P if is_d1 else 0
                bf = j == 0 and qg == 0
                sps = spool.tile([P, 2, QW], F32, tag="s")
                for i in range(2):
                    kt = kt0 + i
                    # write the full streamed window [c0:] even where the
                    # mask will kill it -- exp reads [c0:] and must never
                    # see uninitialized PSUM (fresh-device bits can be NaN)
                    sc0 = c0
                    nc.tensor.matmul(
                        sps[:, i, sc0:],
                        lhsT=kh[:, h, kt * P : (kt + 1) * P],
                        rhs=qh[:, h, qg * QW + sc0 : (qg + 1) * QW],
                        start=True,
                        stop=True,
                    )
                if is_d0:
                    nc.vector.tensor_tensor(
                        sps[:, 0, 0:P], sps[:, 0, 0:P], mask_sb[:, 0:P],
                        op=mybir.AluOpType.add,
                    )
                    nc.vector.tensor_tensor(
                        sps[:, 1, 0 : 2 * P], sps[:, 1, 0 : 2 * P],
                        mask_sb[:, P : 3 * P],
                        op=mybir.AluOpType.add,
                    )
                elif is_d1:
                    nc.vector.tensor_tensor(
                        sps[:, 0, 2 * P : 3 * P], sps[:, 0, 2 * P : 3 * P],
                        mask_sb[:, 0:P],
                        op=mybir.AluOpType.add,
                    )
                    nc.vector.tensor_tensor(
                        sps[:, 1, 2 * P :], sps[:, 1, 2 * P :],
                        mask_sb[:, P : 3 * P],
                        op=mybir.AluOpType.add,
                    )
                if bf:
                    e = elpool.tile([P, 2, QW], BF16, tag="el")
                else:
                    e = ehpool.tile([P, 2, QW], F8, tag="eh")
                nc.scalar.activation(
                    e[:, :, c0:], sps[:, :, c0:],
                    mybir.ActivationFunctionType.Exp, scale=EXP_SCALE,
                )
                last = j == npairs - 1
                if bf:
                    for i in range(2):
                        nc.tensor.matmul(
                            ups,
                            lhsT=v_lo[:, i, rs],
                            rhs=e[:, i, :],
                            start=(i == 0),
                            stop=False,
                        )
                        nc.tensor.matmul(
                            dps,
                            lhsT=ones_b,
                            rhs=e[:, i, :],
                            start=(i == 0),
                            stop=False,
                        )
                else:
                    first = j == 0
                    nc.tensor.matmul(
                        ups[:, c0:],
                        lhsT=v8[:, kt0 : kt0 + 2, rs],
                        rhs=e[:, :, c0:],
                        start=first,
                        stop=last,
                        perf_mode=DR,
                    )
                    nc.tensor.matmul(
                        dps[:, c0:],
                        lhsT=ones8,
                        rhs=e[:, :, c0:],
                        start=first,
                        stop=last,
                        perf_mode=DR,
                    )
            rcp = rpool.tile([P, QW], F32, tag="r")
            nc.vector.reciprocal_approx_fast(rcp, dps)
            nc.vector.tensor_tensor(
                a_sb[:, h, qg * QW : (qg + 1) * QW], ups, rcp,
                op=mybir.AluOpType.mult,
            )

        def proj_mtile(m, ypool, ybpool, last=False):
            """output y rows m*128:(m+1)*128 (all 2048 cols)."""
            yb = ybpool.tile([P, D], BF16, tag="yb")
            for n in range(NQ):
                yp = ypool.tile([P, QW], F32, tag="yp")
                for hh in range(NH):
                    nc.tensor.matmul(
                        yp,
                        lhsT=a_sb[:, hh, m * P : (m + 1) * P],
                        rhs=wp_sb[:, hh, n * QW : (n + 1) * QW],
                        start=(hh == 0),
                        stop=(hh == NH - 1),
                    )
                ns = slice(n * QW, (n + 1) * QW)
                if last and n % 2 == 0:
                    nc.scalar.copy(yb[:, ns], yp)
                else:
                    nc.vector.tensor_copy(yb[:, ns], yp)
            (nc.sync if m % 2 == 0 else nc.gpsimd).dma_start(y[m], yb)

        # ---- emission schedule ----
        for n in range(NQ):
            for h in range(NH):
                qk_dr(h, n)
            for m in range(4 * n, 4 * n + 4):
                if m >= 2:
                    v_block(m)
            if n == 0:
                for m in range(2):
                    v_block(m)
                ph1a.close()
        ph1.close()

        ph2 = ExitStack()
        ph3 = ExitStack()
        spool = ph2.enter_context(tc.tile_pool(name="spsum", bufs=2, space="PSUM"))
        upool = ph2.enter_context(tc.tile_pool(name="upsum", bufs=1, space="PSUM"))
        dpool = ph2.enter_context(tc.tile_pool(name="dpsum", bufs=1, space="PSUM"))
        elpool = ph2.enter_context(tc.tile_pool(name="elpool", bufs=2))
        ehpool = ph2.enter_context(tc.tile_pool(name="ehpool", bufs=4))
        rpool = ph2.enter_context(tc.tile_pool(name="rpool", bufs=2))
        ph2_pools = (spool, elpool, ehpool, upool, dpool, rpool)
        ypool = ph3.enter_context(tc.tile_pool(name="ypsum", bufs=2, space="PSUM"))
        ybpool = ph3.enter_context(tc.tile_pool(name="ybounce", bufs=4))

        # q-group major: after all heads finish q-group qg, y rows
        # 4qg..4qg+3 are fully determined -> the projection matmuls fill
        # the exp-latency gaps of the next q-group.
        for qg in range(NQ):
            for h in range(NH):
                attn_one_qg(h, qg, ph2_pools)
                if qg > 0:
                    proj_mtile(4 * (qg - 1) + h, ypool, ybpool)
        for mi in range(4):
            proj_mtile(12 + mi, ypool, ybpool, last=True)
        ph3.close()
        ph2.close()

    nc.finalize()
    return nc


def _build_mask():
    k = np.arange(P)[:, None]
    c = np.arange(P)[None, :]
    tri = np.where(c >= k, 0.0, MASK_NEG).astype(np.float32)
    dead = np.full((P, P), MASK_NEG, dtype=np.float32)
    return np.concatenate([tri, dead, tri], axis=1)


_NC_CACHE = {}


def _get_nc():
    if "nc" not in _NC_CACHE:
        _NC_CACHE["nc"] = build_bass()
    return _NC_CACHE["nc"]


def make_in_maps(x, W_qkv, W_proj):
    x = np.asarray(x, dtype=np.float32)
    W_qkv = np.asarray(W_qkv, dtype=np.float32)
    W_proj = np.asarray(W_proj, dtype=np.float32)
    Wq, Wk, Wv = W_qkv[0:D], W_qkv[D : 2 * D], W_qkv[2 * D : 3 * D]
    mask = _build_mask()

    def pair8(a):  # [D, M] fp32 -> [P, KJ, 2, M] fp8 (d = j*256+i*128+p)
        a8 = (a * WS).astype(ml_dtypes.float8_e4m3)
        return np.ascontiguousarray(
            a8.reshape(KJ, 2, P, a.shape[1]).transpose(2, 0, 1, 3)
        )

    def kt16(a):  # [D, M] fp32 -> [P, KT, M] bf16 (d = k*128+p), scaled
        ab = (a * WS).astype(ml_dtypes.bfloat16)
        return np.ascontiguousarray(ab.reshape(KT, P, -1).transpose(1, 0, 2))

    # per-batch tensors (shared by 4 cores each)
    xT = [np.ascontiguousarray(x[b].T) for b in range(B)]  # [D, S]
    x8_b = [
        np.ascontiguousarray(
            xT[b]
            .astype(ml_dtypes.float8_e4m3)
            .reshape(KJ, 2, P, NQ, QW)
            .transpose(2, 3, 0, 1, 4)
        )
        for b in range(B)
    ]
    xb0_b = [
        np.ascontiguousarray(
            xT[b][:, 0:BW].astype(ml_dtypes.bfloat16).reshape(KT, P, BW).transpose(1, 0, 2)
        )
        for b in range(B)
    ]
    # per head-group weight slices (shared across batches)
    w_slices = []
    for hg in range(HG):
        rows = slice(hg * NH * HD, (hg + 1) * NH * HD)
        wqT = np.ascontiguousarray(Wq[rows].T)  # [D, 512]
        wkT = np.ascontiguousarray(Wk[rows].T)
        wvT = np.ascontiguousarray(Wv[rows].T)
        wpT = np.ascontiguousarray(W_proj[:, rows].T)  # [512, D]
        w_slices.append(
            {
                "wq8": pair8(wqT),
                "wk8": pair8(wkT),
                "wv8": pair8(wvT),
                "wvb": kt16(wvT),
                "wp": np.ascontiguousarray(
                    (wpT / WS).astype(ml_dtypes.bfloat16).reshape(NH, P, D).transpose(1, 0, 2)
                ),
            }
        )

    in_maps = []
    for c in range(8):
        b, hg = c // HG, c % HG
        m = {"x8": x8_b[b], "xb0": xb0_b[b], "mask": mask}
        m.update(w_slices[hg])
        in_maps.append(m)
    return in_maps


def run(x, W_qkv, W_proj, trace=False):
    nc = _get_nc()
    in_maps = make_in_maps(x, W_qkv, W_proj)
    res = run_bass_kernel_spmd(nc, in_maps, core_ids=list(range(8)), trace=trace)
    out = np.zeros((B, S, D), dtype=np.float32)
    for c in range(8):
        out[c // HG] += res.results[c]["y"].astype(np.float32).reshape(S, D)
    return out, res


def kernel(x, W_qkv, W_proj):
    out, _ = run(x, W_qkv, W_proj, trace=False)
    return out


# revision 31
# speedup vs baseline: 1.0354x; 1.0354x over previous
"""Causal self-attention (B=2, S=2048, D=2048, H=16) on 8 trn2 NeuronCores.

Sharding: core c -> batch b = c//4, head-group hg = c%4 (4 heads of 128 dims).
Each core computes its heads' attention plus the partial output projection
(row-parallel split of W_proj); the host sums the 4 partials per batch.

Mixed-precision, fully SBUF-resident pipeline:
 - QKV projection in fp8e4 with DoubleRow matmuls (two 128-deep k-slices
   contracted per instruction); only V rows < 256 are additionally computed
   in bf16 (short-context queries average over few keys, so their V error
   does not wash out). W_qkv is host-scaled by 32 so fp8 weight entries sit
   in the normal range; q/k/v come out scaled by 32 (wp is host-scaled by
   1/32 to compensate).
 - Scores in bf16 (q,k at 32x -> scores at 1024x; the exp activation scale
   folds the 1/1024 back out). exp is batched per k-tile pair as one ACT
   instruction reading a [128,2,512] PSUM strip.
 - attention@V and the softmax-denominator matmuls run fp8 DoubleRow
   everywhere except (q-group 0, k-tiles 0/1), which stays bf16: short-
   context queries are the precision-critical ones; long-context fp8
   error averages out across keys. The denominator uses a ones *matrix*
   as the stationary operand, so it lands PSUM-replicated across all 128
   partitions and needs no broadcast before the reciprocal multiply.
 - Output projection bf16, interleaved per q-group with head 3's
   attention so PE fills the exp-latency gaps; y written bf16 and summed
   on the host in fp32.
"""

import sys

sys.path.insert(0, "/opt/trn_rl_repo")

from contextlib import ExitStack

import ml_dtypes
import numpy as np

import concourse.bass as bass
import concourse.mybir as mybir
import concourse.tile as tile
from concourse import bacc
from concourse.bass_utils import run_bass_kernel_spmd

B, S, D, H = 2, 2048, 2048, 16
HD = D // H  # 128
NH = 4  # heads per core
HG = H // NH  # head groups = 4
P = 128
KT = D // P  # 16 k-tiles over model dim
KJ = KT // 2  # 8 k-pairs for DoubleRow
NQ = 4  # seq blocks of 512
QW = S // NQ  # 512
ST = S // P  # 16 seq tiles of 128
BW = 256  # bf16 window: seq rows [0, BW) use bf16 QKV / e / v
WS = 32.0  # host-side weight scale for fp8
SCALE = float(1.0 / np.sqrt(D).astype(np.float32))
EXP_SCALE = SCALE / (WS * WS)
MASK_NEG = -1.0e9

F32 = mybir.dt.float32
BF16 = mybir.dt.bfloat16
F8 = mybir.dt.float8e4
DR = mybir.MatmulPerfMode.DoubleRow


def build_bass():
    nc = bacc.Bacc("TRN2")

    # fp8 x, pair layout: d = j*256 + i*128 + p  -> [p, j, i, s]
    x8 = nc.declare_dram_parameter("x8", [P, NQ, KJ, 2, QW], F8, isOutput=False)
    # bf16 x, seq rows [0, 256): d = k*128 + p -> [p, k, s]
    xb0 = nc.declare_dram_parameter("xb0", [P, KT, BW], BF16, isOutput=False)
    wq8 = nc.declare_dram_parameter("wq8", [P, KJ, 2, NH * HD], F8, isOutput=False)
    wk8 = nc.declare_dram_parameter("wk8", [P, KJ, 2, NH * HD], F8, isOutput=False)
    wv8 = nc.declare_dram_parameter("wv8", [P, KJ, 2, NH * HD], F8, isOutput=False)
    wvb = nc.declare_dram_parameter("wvb", [P, KT, NH * HD], BF16, isOutput=False)
    wp = nc.declare_dram_parameter("wp", [P, NH, D], BF16, isOutput=False)
    # mask strip: [:, 0:128] triangular (col>=row allowed), [:, 128:384]
    # dead(128) | triangular(128)
    mask = nc.declare_dram_parameter("mask", [P, 3 * P], F32, isOutput=False)
    y = nc.declare_dram_parameter("y", [ST, P, D], BF16, isOutput=True)

    with tile.TileContext(nc) as tc, ExitStack() as top:
        const = top.enter_context(tc.tile_pool(name="const", bufs=1))
        persist = top.enter_context(tc.tile_pool(name="persist", bufs=1))

        mask_sb = const.tile([P, 3 * P], F32)
        nc.sync.dma_start(mask_sb, mask[:, :])
        ones_b = const.tile([P, P], BF16)
        nc.vector.memset(ones_b, 1.0)
        ones8 = const.tile([P, 2, P], F8)
        nc.vector.memset(ones8, 1.0)

        qh = persist.tile([P, NH, S], BF16)  # Q^T per head (32x scale)
        kh = persist.tile([P, NH, S], BF16)  # K^T per head (32x scale)
        v_lo = persist.tile([P, 2, NH * HD], BF16)  # v rows < 256 (32x)
        v8 = persist.tile([P, ST, NH * HD], F8)  # all v rows, fp8 (32x)
        a_sb = persist.tile([P, NH, S], BF16)  # attention out (32x)
        wp_sb = persist.tile([P, NH, D], BF16)  # W_proj slice (1/32 scale)

        ph1a = ExitStack()  # bf16 inputs: freed after the early bf16 matmuls
        ph1 = ExitStack()   # fp8 inputs: freed after the last DR matmul
        ph1_sb = ph1.enter_context(tc.tile_pool(name="ph1sb", bufs=1, side="right"))
        ph1_ps = ph1.enter_context(
            tc.tile_pool(name="ph1ps", bufs=2, space="PSUM", side="right")
        )
        ph1a_sb = ph1a.enter_context(
            tc.tile_pool(name="ph1asb", bufs=1, side="right")
        )

        xb0_sb = ph1a_sb.tile([P, KT, BW], BF16)
        wvb_sb = ph1a_sb.tile([P, KT, NH * HD], BF16)
        x8_sb = ph1_sb.tile([P, NQ, KJ, 2, QW], F8)
        wq8_sb = ph1_sb.tile([P, KJ, 2, NH * HD], F8)
        wk8_sb = ph1_sb.tile([P, KJ, 2, NH * HD], F8)
        wv8_sb = ph1_sb.tile([P, KJ, 2, NH * HD], F8)

        # --- DMA schedule: three queues, ordered by consumption time ---
        # scalar: fp8 q weights, then fp8 x by seq quarter (j-halved for an
        # early first matmul) -- matches the n-major phase-1 loop.
        nc.scalar.dma_start(wq8_sb[:, 0:2, :, :], wq8[:, 0:2, :, :])
        nc.scalar.dma_start(x8_sb[:, 0, 0:2, :, :], x8[:, 0, 0:2, :, :])
        nc.scalar.dma_start(wq8_sb[:, 2:8, :, :], wq8[:, 2:8, :, :])
        nc.scalar.dma_start(x8_sb[:, 0, 2:8, :, :], x8[:, 0, 2:8, :, :])
        for n in range(1, NQ):
            nc.scalar.dma_start(x8_sb[:, n, 0:4, :, :], x8[:, n, 0:4, :, :])
            nc.scalar.dma_start(x8_sb[:, n, 4:8, :, :], x8[:, n, 4:8, :, :])
        # gpsimd: fp8 k weights, fp8 v weights, then W_proj
        nc.gpsimd.dma_start(wk8_sb[:, 0:2, :, :], wk8[:, 0:2, :, :])
        nc.gpsimd.dma_start(wk8_sb[:, 2:8, :, :], wk8[:, 2:8, :, :])
        nc.gpsimd.dma_start(wv8_sb, wv8[:, :, :, :])
        nc.gpsimd.dma_start(wp_sb, wp[:, :, :])
        # sync: mask, bf16 x block, bf16 v weights (all for the small bf16
        # v pass, consumed ~40us in)
        for kc in range(0, KT, 4):
            ks = slice(kc, kc + 4)
            nc.sync.dma_start(xb0_sb[:, ks, :], xb0[:, ks, :])
            nc.sync.dma_start(wvb_sb[:, ks, :], wvb[:, ks, :])

        def qk_dr(h, n):
            """fp8 DoubleRow q,k for head h, seq block n."""
            cs = slice(h * HD, (h + 1) * HD)
            ss = slice(n * QW, (n + 1) * QW)
            for w_sb, out in ((wq8_sb, qh), (wk8_sb, kh)):
                ps = ph1_ps.tile([P, QW], F32, tag="ps")
                for j in range(KJ):
                    nc.tensor.matmul(
                        ps,
                        lhsT=w_sb[:, j, :, cs],
                        rhs=x8_sb[:, n, j, :, :],
                        start=(j == 0),
                        stop=(j == KJ - 1),
                        perf_mode=DR,
                    )
                nc.vector.tensor_copy(out[:, h, ss], ps)

        def v_block(m):
            """v rows m*128:(m+1)*128 (all 4 heads)."""
            ms = slice(m * P, (m + 1) * P)
            ps = ph1_ps.tile([P, QW], F32, tag="ps")
            if m < 2:
                for k in range(KT):
                    nc.tensor.matmul(
                        ps,
                        lhsT=xb0_sb[:, k, ms],
                        rhs=wvb_sb[:, k, :],
                        start=(k == 0),
                        stop=(k == KT - 1),
                    )
                nc.vector.tensor_copy(v_lo[:, m, :], ps)
                nc.vector.tensor_copy(v8[:, m, :], ps)
            else:
                for j in range(KJ):
                    nc.tensor.matmul(
                        ps,
                        lhsT=x8_sb[:, m // 4, j, :, (m % 4) * P : (m % 4 + 1) * P],
                        rhs=wv8_sb[:, j, :, :],
                        start=(j == 0),
                        stop=(j == KJ - 1),
                        perf_mode=DR,
                    )
                nc.vector.tensor_copy(v8[:, m, :], ps)

        def attn_one_qg(h, qg, pools):
            spool, elpool, ehpool, upool, dpool, rpool = pools
            rs = slice(h * HD, (h + 1) * HD)
            kmax = 4 * qg + 4
            npairs = kmax // 2
            ups = upool.tile([P, QW], F32, tag="u")
            dps = dpool.tile([P, QW], F32, tag="d")
            for j in range(npairs):
                kt0 = 2 * j
                is_d0 = j == 2 * qg
                is_d1 = j == 2 * qg + 1
                c0 = 2 * P if is_d1 else 0
                bf = j == 0 and qg == 0
                sps = spool.tile([P, 2, QW], F32, tag="s")
                for i in range(2):
                    kt = kt0 + i
                    # write the full streamed window [c0:] even where the
                    # mask will kill it -- exp reads [c0:] and must never
                    # see uninitialized PSUM (fresh-device bits can be NaN)
                    sc0 = c0
                    nc.tensor.matmul(
                        sps[:, i, sc0:],
                        lhsT=kh[:, h, kt * P : (kt + 1) * P],
                        rhs=qh[:, h, qg * QW + sc0 : (qg + 1) * QW],
                        start=True,
                        stop=True,
                    )
                if is_d0:
                    nc.vector.tensor_tensor(
                        sps[:, 0, 0:P], sps[:, 0, 0:P], mask_sb[:, 0:P],
                        op=mybir.AluOpType.add,
                    )
                    nc.vector.tensor_tensor(
                        sps[:, 1, 0 : 2 * P], sps[:, 1, 0 : 2 * P],
                        mask_sb[:, P : 3 * P],
                        op=mybir.AluOpType.add,
                    )
                elif is_d1:
                    nc.vector.tensor_tensor(
                        sps[:, 0, 2 * P : 3 * P], sps[:, 0, 2 * P : 3 * P],
                        mask_sb[:, 0:P],
                        op=mybir.AluOpType.add,
                    )
                    nc.vector.tensor_tensor(
                        sps[:, 1, 2 * P :], sps[:, 1, 2 * P :],
                        mask_sb[:, P : 3 * P],
                        op=mybir.AluOpType.add,
                    )
                if bf:
                    e = elpool.tile([P, 2, QW], BF16, tag="el")
                else:
                    e = ehpool.tile([P, 2, QW], F8, tag="eh")
                nc.scalar.activation(
                    e[:, :, c0:], sps[:, :, c0:],
                    mybir.ActivationFunctionType.Exp, scale=EXP_SCALE,
                )
                last = j == npairs - 1
                if bf:
                    for i in range(2):
                        nc.tensor.matmul(
                            ups,
                            lhsT=v_lo[:, i, rs],
                            rhs=e[:, i, :],
                            start=(i == 0),
                            stop=False,
                        )
                        nc.tensor.matmul(
                            dps,
                            lhsT=ones_b,
                            rhs=e[:, i, :],
                            start=(i == 0),
                            stop=False,
                        )
                else:
                    first = j == 0
                    nc.tensor.matmul(
                        ups[:, c0:],
                        lhsT=v8[:, kt0 : kt0 + 2, rs],
                        rhs=e[:, :, c0:],
                        start=first,
                        stop=last,
                        perf_mode=DR,
                    )
                    nc.tensor.matmul(
                        dps[:, c0:],
                        lhsT=ones8,
                        rhs=e[:, :, c0:],
                        start=first,
                        stop=last,
                        perf_mode=DR,
                    )
            rcp = rpool.tile([P, QW], F32, tag="r")
            nc.vector.reciprocal_approx_fast(rcp, dps)
            nc.vector.tensor_tensor(
                a_sb[:, h, qg * QW : (qg + 1) * QW], ups, rcp,
                op=mybir.AluOpType.mult,
            )

        def proj_mtile(m, ypool, ybpool, last=False):
            """output y rows m*128:(m+1)*128 (all 2048 cols)."""
            yb = ybpool.tile([P, D], BF16, tag="yb")
            for n in range(NQ):
                yp = ypool.tile([P, QW], F32, tag="yp")
                for hh in range(NH):
                    nc.tensor.matmul(
                        yp,
                        lhsT=a_sb[:, hh, m * P : (m + 1) * P],
                        rhs=wp_sb[:, hh, n * QW : (n + 1) * QW],
                        start=(hh == 0),
                        stop=(hh == NH - 1),
                    )
                ns = slice(n * QW, (n + 1) * QW)
                if last and n % 2 == 0:
                    nc.scalar.copy(yb[:, ns], yp)
                else:
                    nc.vector.tensor_copy(yb[:, ns], yp)
            (nc.sync if m % 2 == 0 else nc.gpsimd).dma_start(y[m], yb)

        # ---- emission schedule ----
        for n in range(NQ):
            for h in range(NH):
                qk_dr(h, n)
            for m in range(4 * n, 4 * n + 4):
                if m >= 2:
                    v_block(m)
            if n == 0:
                for m in range(2):
                    v_block(m)
                ph1a.close()
        ph1.close()

        ph2 = ExitStack()
        ph3 = ExitStack()
        spool = ph2.enter_context(tc.tile_pool(name="spsum", bufs=2, space="PSUM"))
        upool = ph2.enter_context(tc.tile_pool(name="upsum", bufs=1, space="PSUM"))
        dpool = ph2.enter_context(tc.tile_pool(name="dpsum", bufs=1, space="PSUM"))
        elpool = ph2.enter_context(tc.tile_pool(name="elpool", bufs=2))
        ehpool = ph2.enter_context(tc.tile_pool(name="ehpool", bufs=4))
        rpool = ph2.enter_context(tc.tile_pool(name="rpool", bufs=2))
        ph2_pools = (spool, elpool, ehpool, upool, dpool, rpool)
        ypool = ph3.enter_context(tc.tile_pool(name="ypsum", bufs=2, space="PSUM"))
        ybpool = ph3.enter_context(tc.tile_pool(name="ybounce", bufs=4))

        # q-group major: after all heads finish q-group qg, y rows
        # 4qg..4qg+3 are fully determined -> the projection matmuls fill
        # the exp-latency gaps of the next q-group.
        for qg in range(NQ):
            for h in range(NH):
                attn_one_qg(h, qg, ph2_pools)
                if qg > 0:
                    proj_mtile(4 * (qg - 1) + h, ypool, ybpool)
        for mi in range(4):
            proj_mtile(12 + mi, ypool, ybpool, last=True)
        ph3.close()
        ph2.close()

    nc.finalize()
    return nc


def _build_mask():
    k = np.arange(P)[:, None]
    c = np.arange(P)[None, :]
    tri = np.where(c >= k, 0.0, MASK_NEG).astype(np.float32)
    dead = np.full((P, P), MASK_NEG, dtype=np.float32)
    return np.concatenate([tri, dead, tri], axis=1)


_NC_CACHE = {}


def _get_nc():
    if "nc" not in _NC_CACHE:
        _NC_CACHE["nc"] = build_bass()
    return _NC_CACHE["nc"]


def make_in_maps(x, W_qkv, W_proj):
    x = np.asarray(x, dtype=np.float32)
    W_qkv = np.asarray(W_qkv, dtype=np.float32)
    W_proj = np.asarray(W_proj, dtype=np.float32)
    Wq, Wk, Wv = W_qkv[0:D], W_qkv[D : 2 * D], W_qkv[2 * D : 3 * D]
    mask = _build_mask()

    def pair8(a):  # [D, M] fp32 -> [P, KJ, 2, M] fp8 (d = j*256+i*128+p)
        a8 = (a * WS).astype(ml_dtypes.float8_e4m3)
        return np.ascontiguousarray(
            a8.reshape(KJ, 2, P, a.shape[1]).transpose(2, 0, 1, 3)
        )

    def kt16(a):  # [D, M] fp32 -> [P, KT, M] bf16 (d = k*128+p), scaled
        ab = (a * WS).astype(ml_dtypes.bfloat16)
        return np.ascontiguousarray(ab.reshape(KT, P, -1).transpose(1, 0, 2))

    # per-batch tensors (shared by 4 cores each)
    xT = [np.ascontiguousarray(x[b].T) for b in range(B)]  # [D, S]
    x8_b = [
        np.ascontiguousarray(
            xT[b]
            .astype(ml_dtypes.float8_e4m3)
            .reshape(KJ, 2, P, NQ, QW)
            .transpose(2, 3, 0, 1, 4)
        )
        for b in range(B)
    ]
    xb0_b = [
        np.ascontiguousarray(
            xT[b][:, 0:BW].astype(ml_dtypes.bfloat16).reshape(KT, P, BW).transpose(1, 0, 2)
        )
        for b in range(B)
    ]
    # per head-group weight slices (shared across batches)
    w_slices = []
    for hg in range(HG):
        rows = slice(hg * NH * HD, (hg + 1) * NH * HD)
        wqT = np.ascontiguousarray(Wq[rows].T)  # [D, 512]
        wkT = np.ascontiguousarray(Wk[rows].T)
        wvT = np.ascontiguousarray(Wv[rows].T)
        wpT = np.ascontiguousarray(W_proj[:, rows].T)  # [512, D]
        w_slices.append(
            {
                "wq8": pair8(wqT),
                "wk8": pair8(wkT),
                "wv8": pair8(wvT),
                "wvb": kt16(wvT),
                "wp": np.ascontiguousarray(
                    (wpT / WS).astype(ml_dtypes.bfloat16).reshape(NH, P, D).transpose(1, 0, 2)
                ),
            }
        )

    in_maps = []
    for c in range(8):
        b, hg = c // HG, c % HG
        m = {"x8": x8_b[b], "xb0": xb0_b[b], "mask": mask}
        m.update(w_slices[hg])
        in_maps.append(m)
    return in_maps


def run(x, W_qkv, W_proj, trace=False):
    nc = _get_nc()
    in_maps = make_in_maps(x, W_qkv, W_proj)
    res = run_bass_kernel_spmd(nc, in_maps, core_ids=list(range(8)), trace=trace)
    out = np.zeros((B, S, D), dtype=np.float32)
    for c in range(8):
        out[c // HG] += res.results[c]["y"].astype(np.float32).reshape(S, D)
    return out, res


def kernel(x, W_qkv, W_proj):
    out, _ = run(x, W_qkv, W_proj, trace=False)
    return out


# revision 32
# speedup vs baseline: 1.0357x; 1.0002x over previous
"""Causal self-attention (B=2, S=2048, D=2048, H=16) on 8 trn2 NeuronCores.

Sharding: core c -> batch b = c//4, head-group hg = c%4 (4 heads of 128 dims).
Each core computes its heads' attention plus the partial output projection
(row-parallel split of W_proj); the host sums the 4 partials per batch.

Mixed-precision, fully SBUF-resident pipeline:
 - QKV projection in fp8e4 with DoubleRow matmuls (two 128-deep k-slices
   contracted per instruction); only V rows < 256 are additionally computed
   in bf16 (short-context queries average over few keys, so their V error
   does not wash out). W_qkv is host-scaled by 32 so fp8 weight entries sit
   in the normal range; q/k/v come out scaled by 32 (wp is host-scaled by
   1/32 to compensate).
 - Scores in bf16 (q,k at 32x -> scores at 1024x; the exp activation scale
   folds the 1/1024 back out). exp is batched per k-tile pair as one ACT
   instruction reading a [128,2,512] PSUM strip.
 - attention@V and the softmax-denominator matmuls run fp8 DoubleRow
   everywhere except (q-group 0, k-tiles 0/1), which stays bf16: short-
   context queries are the precision-critical ones; long-context fp8
   error averages out across keys. The denominator uses a ones *matrix*
   as the stationary operand, so it lands PSUM-replicated across all 128
   partitions and needs no broadcast before the reciprocal multiply.
 - Output projection bf16, interleaved per q-group with head 3's
   attention so PE fills the exp-latency gaps; y written bf16 and summed
   on the host in fp32.
"""

import sys

sys.path.insert(0, "/opt/trn_rl_repo")

from contextlib import ExitStack

import ml_dtypes
import numpy as np

import concourse.bass as bass
import concourse.mybir as mybir
import concourse.tile as tile
from concourse import bacc
from concourse.bass_utils import run_bass_kernel_spmd

B, S, D, H = 2, 2048, 2048, 16
HD = D // H  # 128
NH = 4  # heads per core
HG = H // NH  # head groups = 4
P = 128
KT = D // P  # 16 k-tiles over model dim
KJ = KT // 2  # 8 k-pairs for DoubleRow
NQ = 4  # seq blocks of 512
QW = S // NQ  # 512
ST = S // P  # 16 seq tiles of 128
BW = 256  # bf16 window: seq rows [0, BW) use bf16 QKV / e / v
WS = 32.0  # host-side weight scale for fp8
SCALE = float(1.0 / np.sqrt(D).astype(np.float32))
EXP_SCALE = SCALE / (WS * WS)
MASK_NEG = -1.0e9

F32 = mybir.dt.float32
BF16 = mybir.dt.bfloat16
F8 = mybir.dt.float8e4
DR = mybir.MatmulPerfMode.DoubleRow


def build_bass():
    nc = bacc.Bacc("TRN2")

    # fp8 x, pair layout: d = j*256 + i*128 + p  -> [p, j, i, s]
    x8 = nc.declare_dram_parameter("x8", [P, NQ, KJ, 2, QW], F8, isOutput=False)
    # bf16 x, seq rows [0, 256): d = k*128 + p -> [p, k, s]
    xb0 = nc.declare_dram_parameter("xb0", [P, KT, BW], BF16, isOutput=False)
    wq8 = nc.declare_dram_parameter("wq8", [P, KJ, 2, NH * HD], F8, isOutput=False)
    wk8 = nc.declare_dram_parameter("wk8", [P, KJ, 2, NH * HD], F8, isOutput=False)
    wv8 = nc.declare_dram_parameter("wv8", [P, KJ, 2, NH * HD], F8, isOutput=False)
    wvb = nc.declare_dram_parameter("wvb", [P, KT, NH * HD], BF16, isOutput=False)
    wp = nc.declare_dram_parameter("wp", [P, NH, D], BF16, isOutput=False)
    # mask strip: [:, 0:128] triangular (col>=row allowed), [:, 128:384]
    # dead(128) | triangular(128)
    mask = nc.declare_dram_parameter("mask", [P, 3 * P], F32, isOutput=False)
    y = nc.declare_dram_parameter("y", [ST, P, D], BF16, isOutput=True)

    with tile.TileContext(nc) as tc, ExitStack() as top:
        const = top.enter_context(tc.tile_pool(name="const", bufs=1))
        persist = top.enter_context(tc.tile_pool(name="persist", bufs=1))

        mask_sb = const.tile([P, 3 * P], F32)
        nc.sync.dma_start(mask_sb, mask[:, :])
        ones_b = const.tile([P, P], BF16)
        nc.vector.memset(ones_b, 1.0)
        ones8 = const.tile([P, 2, P], F8)
        nc.vector.memset(ones8, 1.0)

        qh = persist.tile([P, NH, S], BF16)  # Q^T per head (32x scale)
        kh = persist.tile([P, NH, S], BF16)  # K^T per head (32x scale)
        v_lo = persist.tile([P, 2, NH * HD], BF16)  # v rows < 256 (32x)
        v8 = persist.tile([P, ST, NH * HD], F8)  # all v rows, fp8 (32x)
        a_sb = persist.tile([P, NH, S], BF16)  # attention out (32x)
        wp_sb = persist.tile([P, NH, D], BF16)  # W_proj slice (1/32 scale)

        ph1a = ExitStack()  # bf16 inputs: freed after the early bf16 matmuls
        ph1 = ExitStack()   # fp8 inputs: freed after the last DR matmul
        ph1_sb = ph1.enter_context(tc.tile_pool(name="ph1sb", bufs=1, side="right"))
        ph1_ps = ph1.enter_context(
            tc.tile_pool(name="ph1ps", bufs=2, space="PSUM", side="right")
        )
        ph1a_sb = ph1a.enter_context(
            tc.tile_pool(name="ph1asb", bufs=1, side="right")
        )

        xb0_sb = ph1a_sb.tile([P, KT, BW], BF16)
        wvb_sb = ph1a_sb.tile([P, KT, NH * HD], BF16)
        x8_sb = ph1_sb.tile([P, NQ, KJ, 2, QW], F8)
        wq8_sb = ph1_sb.tile([P, KJ, 2, NH * HD], F8)
        wk8_sb = ph1_sb.tile([P, KJ, 2, NH * HD], F8)
        wv8_sb = ph1_sb.tile([P, KJ, 2, NH * HD], F8)

        # --- DMA schedule: three queues, ordered by consumption time ---
        # scalar: fp8 q weights, then fp8 x by seq quarter (j-halved for an
        # early first matmul) -- matches the n-major phase-1 loop.
        nc.scalar.dma_start(wq8_sb[:, 0:2, :, :], wq8[:, 0:2, :, :])
        nc.scalar.dma_start(x8_sb[:, 0, 0:2, :, :], x8[:, 0, 0:2, :, :])
        nc.scalar.dma_start(wq8_sb[:, 2:8, :, :], wq8[:, 2:8, :, :])
        nc.scalar.dma_start(x8_sb[:, 0, 2:8, :, :], x8[:, 0, 2:8, :, :])
        nc.scalar.dma_start(x8_sb[:, 1, 0:4, :, :], x8[:, 1, 0:4, :, :])
        nc.scalar.dma_start(x8_sb[:, 1, 4:8, :, :], x8[:, 1, 4:8, :, :])
        # gpsimd: fp8 k weights, fp8 v weights, then W_proj
        nc.gpsimd.dma_start(wk8_sb[:, 0:2, :, :], wk8[:, 0:2, :, :])
        nc.gpsimd.dma_start(wk8_sb[:, 2:8, :, :], wk8[:, 2:8, :, :])
        nc.gpsimd.dma_start(wv8_sb, wv8[:, :, :, :])
        nc.gpsimd.dma_start(x8_sb[:, 3, :, :, :], x8[:, 3, :, :, :])
        nc.gpsimd.dma_start(wp_sb, wp[:, :, :])
        # sync: mask, bf16 x block, bf16 v weights (all for the small bf16
        # v pass, consumed ~40us in)
        for kc in range(0, KT, 4):
            ks = slice(kc, kc + 4)
            nc.sync.dma_start(xb0_sb[:, ks, :], xb0[:, ks, :])
            nc.sync.dma_start(wvb_sb[:, ks, :], wvb[:, ks, :])
        nc.sync.dma_start(x8_sb[:, 2, :, :, :], x8[:, 2, :, :, :])

        def qk_dr(h, n):
            """fp8 DoubleRow q,k for head h, seq block n."""
            cs = slice(h * HD, (h + 1) * HD)
            ss = slice(n * QW, (n + 1) * QW)
            for w_sb, out in ((wq8_sb, qh), (wk8_sb, kh)):
                ps = ph1_ps.tile([P, QW], F32, tag="ps")
                for j in range(KJ):
                    nc.tensor.matmul(
                        ps,
                        lhsT=w_sb[:, j, :, cs],
                        rhs=x8_sb[:, n, j, :, :],
                        start=(j == 0),
                        stop=(j == KJ - 1),
                        perf_mode=DR,
                    )
                nc.vector.tensor_copy(out[:, h, ss], ps)

        def v_block(m):
            """v rows m*128:(m+1)*128 (all 4 heads)."""
            ms = slice(m * P, (m + 1) * P)
            ps = ph1_ps.tile([P, QW], F32, tag="ps")
            if m < 2:
                for k in range(KT):
                    nc.tensor.matmul(
                        ps,
                        lhsT=xb0_sb[:, k, ms],
                        rhs=wvb_sb[:, k, :],
                        start=(k == 0),
                        stop=(k == KT - 1),
                    )
                nc.vector.tensor_copy(v_lo[:, m, :], ps)
                nc.vector.tensor_copy(v8[:, m, :], ps)
            else:
                for j in range(KJ):
                    nc.tensor.matmul(
                        ps,
                        lhsT=x8_sb[:, m // 4, j, :, (m % 4) * P : (m % 4 + 1) * P],
                        rhs=wv8_sb[:, j, :, :],
                        start=(j == 0),
                        stop=(j == KJ - 1),
                        perf_mode=DR,
                    )
                nc.vector.tensor_copy(v8[:, m, :], ps)

        def attn_one_qg(h, qg, pools):
            spool, elpool, ehpool, upool, dpool, rpool = pools
            rs = slice(h * HD, (h + 1) * HD)
            kmax = 4 * qg + 4
            npairs = kmax // 2
            ups = upool.tile([P, QW], F32, tag="u")
            dps = dpool.tile([P, QW], F32, tag="d")
            for j in range(npairs):
                kt0 = 2 * j
                is_d0 = j == 2 * qg
                is_d1 = j == 2 * qg + 1
                c0 = 2 * P if is_d1 else 0
                bf = j == 0 and qg == 0
                sps = spool.tile([P, 2, QW], F32, tag="s")
                for i in range(2):
                    kt = kt0 + i
                    # write the full streamed window [c0:] even where the
                    # mask will kill it -- exp reads [c0:] and must never
                    # see uninitialized PSUM (fresh-device bits can be NaN)
                    sc0 = c0
                    nc.tensor.matmul(
                        sps[:, i, sc0:],
                        lhsT=kh[:, h, kt * P : (kt + 1) * P],
                        rhs=qh[:, h, qg * QW + sc0 : (qg + 1) * QW],
                        start=True,
                        stop=True,
                    )
                if is_d0:
                    nc.vector.tensor_tensor(
                        sps[:, 0, 0:P], sps[:, 0, 0:P], mask_sb[:, 0:P],
                        op=mybir.AluOpType.add,
                    )
                    nc.vector.tensor_tensor(
                        sps[:, 1, 0 : 2 * P], sps[:, 1, 0 : 2 * P],
                        mask_sb[:, P : 3 * P],
                        op=mybir.AluOpType.add,
                    )
                elif is_d1:
                    nc.vector.tensor_tensor(
                        sps[:, 0, 2 * P : 3 * P], sps[:, 0, 2 * P : 3 * P],
                        mask_sb[:, 0:P],
                        op=mybir.AluOpType.add,
                    )
                    nc.vector.tensor_tensor(
                        sps[:, 1, 2 * P :], sps[:, 1, 2 * P :],
                        mask_sb[:, P : 3 * P],
                        op=mybir.AluOpType.add,
                    )
                if bf:
                    e = elpool.tile([P, 2, QW], BF16, tag="el")
                else:
                    e = ehpool.tile([P, 2, QW], F8, tag="eh")
                nc.scalar.activation(
                    e[:, :, c0:], sps[:, :, c0:],
                    mybir.ActivationFunctionType.Exp, scale=EXP_SCALE,
                )
                last = j == npairs - 1
                if bf:
                    for i in range(2):
                        nc.tensor.matmul(
                            ups,
                            lhsT=v_lo[:, i, rs],
                            rhs=e[:, i, :],
                            start=(i == 0),
                            stop=False,
                        )
                        nc.tensor.matmul(
                            dps,
                            lhsT=ones_b,
                            rhs=e[:, i, :],
                            start=(i == 0),
                            stop=False,
                        )
                else:
                    first = j == 0
                    nc.tensor.matmul(
                        ups[:, c0:],
                        lhsT=v8[:, kt0 : kt0 + 2, rs],
                        rhs=e[:, :, c0:],
                        start=first,
                        stop=last,
                        perf_mode=DR,
                    )
                    nc.tensor.matmul(
                        dps[:, c0:],
                        lhsT=ones8,
                        rhs=e[:, :, c0:],
                        start=first,
                        stop=last,
                        perf_mode=DR,
                    )
            rcp = rpool.tile([P, QW], F32, tag="r")
            nc.vector.reciprocal_approx_fast(rcp, dps)
            nc.vector.tensor_tensor(
                a_sb[:, h, qg * QW : (qg + 1) * QW], ups, rcp,
                op=mybir.AluOpType.mult,
            )

        def proj_mtile(m, ypool, ybpool, last=False):
            """output y rows m*128:(m+1)*128 (all 2048 cols)."""
            yb = ybpool.tile([P, D], BF16, tag="yb")
            for n in range(NQ):
                yp = ypool.tile([P, QW], F32, tag="yp")
                for hh in range(NH):
                    nc.tensor.matmul(
                        yp,
                        lhsT=a_sb[:, hh, m * P : (m + 1) * P],
                        rhs=wp_sb[:, hh, n * QW : (n + 1) * QW],
                        start=(hh == 0),
                        stop=(hh == NH - 1),
                    )
                ns = slice(n * QW, (n + 1) * QW)
                if last and n % 2 == 0:
                    nc.scalar.copy(yb[:, ns], yp)
                else:
                    nc.vector.tensor_copy(yb[:, ns], yp)
            (nc.sync if m % 2 == 0 else nc.gpsimd).dma_start(y[m], yb)

        # ---- emission schedule ----
        for n in range(NQ):
            for h in range(NH):
                qk_dr(h, n)
            for m in range(4 * n, 4 * n + 4):
                if m >= 2:
                    v_block(m)
            if n == 0:
                for m in range(2):
                    v_block(m)
                ph1a.close()
        ph1.close()

        ph2 = ExitStack()
        ph3 = ExitStack()
        spool = ph2.enter_context(tc.tile_pool(name="spsum", bufs=2, space="PSUM"))
        upool = ph2.enter_context(tc.tile_pool(name="upsum", bufs=1, space="PSUM"))
        dpool = ph2.enter_context(tc.tile_pool(name="dpsum", bufs=1, space="PSUM"))
        elpool = ph2.enter_context(tc.tile_pool(name="elpool", bufs=2))
        ehpool = ph2.enter_context(tc.tile_pool(name="ehpool", bufs=4))
        rpool = ph2.enter_context(tc.tile_pool(name="rpool", bufs=2))
        ph2_pools = (spool, elpool, ehpool, upool, dpool, rpool)
        ypool = ph3.enter_context(tc.tile_pool(name="ypsum", bufs=2, space="PSUM"))
        ybpool = ph3.enter_context(tc.tile_pool(name="ybounce", bufs=4))

        # q-group major: after all heads finish q-group qg, y rows
        # 4qg..4qg+3 are fully determined -> the projection matmuls fill
        # the exp-latency gaps of the next q-group.
        for qg in range(NQ):
            for h in range(NH):
                attn_one_qg(h, qg, ph2_pools)
                if qg > 0:
                    proj_mtile(4 * (qg - 1) + h, ypool, ybpool)
        for mi in range(4):
            proj_mtile(12 + mi, ypool, ybpool, last=True)
        ph3.close()
        ph2.close()

    nc.finalize()
    return nc


def _build_mask():
    k = np.arange(P)[:, None]
    c = np.arange(P)[None, :]
    tri = np.where(c >= k, 0.0, MASK_NEG).astype(np.float32)
    dead = np.full((P, P), MASK_NEG, dtype=np.float32)
    return np.concatenate([tri, dead, tri], axis=1)


_NC_CACHE = {}


def _get_nc():
    if "nc" not in _NC_CACHE:
        _NC_CACHE["nc"] = build_bass()
    return _NC_CACHE["nc"]


def make_in_maps(x, W_qkv, W_proj):
    x = np.asarray(x, dtype=np.float32)
    W_qkv = np.asarray(W_qkv, dtype=np.float32)
    W_proj = np.asarray(W_proj, dtype=np.float32)
    Wq, Wk, Wv = W_qkv[0:D], W_qkv[D : 2 * D], W_qkv[2 * D : 3 * D]
    mask = _build_mask()

    def pair8(a):  # [D, M] fp32 -> [P, KJ, 2, M] fp8 (d = j*256+i*128+p)
        a8 = (a * WS).astype(ml_dtypes.float8_e4m3)
        return np.ascontiguousarray(
            a8.reshape(KJ, 2, P, a.shape[1]).transpose(2, 0, 1, 3)
        )

    def kt16(a):  # [D, M] fp32 -> [P, KT, M] bf16 (d = k*128+p), scaled
        ab = (a * WS).astype(ml_dtypes.bfloat16)
        return np.ascontiguousarray(ab.reshape(KT, P, -1).transpose(1, 0, 2))

    # per-batch tensors (shared by 4 cores each)
    xT = [np.ascontiguousarray(x[b].T) for b in range(B)]  # [D, S]
    x8_b = [
        np.ascontiguousarray(
            xT[b]
            .astype(ml_dtypes.float8_e4m3)
            .reshape(KJ, 2, P, NQ, QW)
            .transpose(2, 3, 0, 1, 4)
        )
        for b in range(B)
    ]
    xb0_b = [
        np.ascontiguousarray(
            xT[b][:, 0:BW].astype(ml_dtypes.bfloat16).reshape(KT, P, BW).transpose(1, 0, 2)
        )
        for b in range(B)
    ]
    # per head-group weight slices (shared across batches)
    w_slices = []
    for hg in range(HG):
        rows = slice(hg * NH * HD, (hg + 1) * NH * HD)
        wqT = np.ascontiguousarray(Wq[rows].T)  # [D, 512]
        wkT = np.ascontiguousarray(Wk[rows].T)
        wvT = np.ascontiguousarray(Wv[rows].T)
        wpT = np.ascontiguousarray(W_proj[:, rows].T)  # [512, D]
        w_slices.append(
            {
                "wq8": pair8(wqT),
                "wk8": pair8(wkT),
                "wv8": pair8(wvT),
                "wvb": kt16(wvT),
                "wp": np.ascontiguousarray(
                    (wpT / WS).astype(ml_dtypes.bfloat16).reshape(NH, P, D).transpose(1, 0, 2)
                ),
            }
        )

    in_maps = []
    for c in range(8):
        b, hg = c // HG, c % HG
        m = {"x8": x8_b[b], "xb0": xb0_b[b], "mask": mask}
        m.update(w_slices[hg])
        in_maps.append(m)
    return in_maps


def run(x, W_qkv, W_proj, trace=False):
    nc = _get_nc()
    in_maps = make_in_maps(x, W_qkv, W_proj)
    res = run_bass_kernel_spmd(nc, in_maps, core_ids=list(range(8)), trace=trace)
    out = np.zeros((B, S, D), dtype=np.float32)
    for c in range(8):
        out[c // HG] += res.results[c]["y"].astype(np.float32).reshape(S, D)
    return out, res


def kernel(x, W_qkv, W_proj):
    out, _ = run(x, W_qkv, W_proj, trace=False)
    return out
